# revision 7
# baseline (speedup 1.0000x reference)
"""Distributed 2-layer GCN + mean-pool + linear + sigmoid on 8 TRN2 NeuronCores.

Sharding: nodes (and their in-edges) are sharded across 8 cores by contiguous
dst ranges. Weights replicated. h1 is AllGathered between layers; per-graph
pooled sums are AllGathered and combined on every core.

Aggregation strategy (aggregate-then-transform, GCN is linear so this is
exact): for each 128-node window, gather the x/h rows of in-edge sources
(fast int16 dma_gather from host-compacted tables), build a sparse scatter
matrix S[e, n] = norm_e * onehot(dstloc_e) on DVE (iota + fused is_equal/mult),
and accumulate aggT[d, n] += Xg^T @ S on the TensorEngine.  Self-loops are a
per-window diagonal matmul.  Bias is a K=1 ones-row matmul into the same PSUM
accumulation; relu (and the 1/cnt pooling scale in layer 2) is one fused DVE
tensor_scalar op.
"""

import math
from contextlib import ExitStack

import numpy as np

P = 128
IN_DIM = 128
HID = 256
SACALL = 1024  # max gather call size (64 desc/engine packet limit)


def _sa_calls(SAK):
    """Stage-A call plan: sizes (each %128==0, <=1024) with offsets."""
    plan, off = [], 0
    while off < SAK:
        L = min(SACALL, SAK - off)
        plan.append((off, L))
        off += L
    return plan


# ----------------------------------------------------------------- host prep


def _snake_pack(node_ids, degs, n_bins):
    """Assign nodes to n_bins bins of <=128 nodes, balancing sum of degs.
    Returns window id and position-in-window per node (dense arrays)."""
    order = np.argsort(-degs, kind="stable")
    nb = len(node_ids)
    win = np.empty(nb, np.int32)
    pos = np.empty(nb, np.int32)
    counts = np.zeros(n_bins, np.int32)
    # snake order: 0..n_bins-1, n_bins-1..0, ...
    fwd = np.arange(n_bins)
    snake = np.concatenate([fwd, fwd[::-1]])
    for i, oi in enumerate(order):
        b = snake[i % (2 * n_bins)]
        # bins fill evenly since nb <= 128*n_bins and snake is balanced
        while counts[b] >= P:
            b = (b + 1) % n_bins
        win[oi] = b
        pos[oi] = counts[b]
        counts[b] += 1
    return win, pos


def _prep(x, edge_index, batch, n_graphs, n_cores):
    N = x.shape[0]
    E = edge_index.shape[1]
    NPC = N // n_cores
    W = math.ceil(NPC / P)
    NSLOT = W * P

    src = np.asarray(edge_index[0], np.int64)
    dst = np.asarray(edge_index[1], np.int64)
    batch = np.asarray(batch, np.int64)

    deg = (np.bincount(dst, minlength=N) + 1).astype(np.float32)
    dinv = (1.0 / np.sqrt(deg)).astype(np.float32)
    norm_e = (dinv[src] * dinv[dst]).astype(np.float32)
    selfw = (dinv * dinv).astype(np.float32)
    cnt = np.bincount(batch, minlength=n_graphs).astype(np.float32)
    icnt_g = (1.0 / np.maximum(cnt, 1.0)).astype(np.float32)

    core_of = dst // NPC
    core_of_node = np.arange(N) // NPC

    # pass 1: per-core window packing of own nodes -> global slot map
    phys = np.empty(N, np.int64)  # node -> row in h_full
    win_of = np.empty(N, np.int32)
    pos_of = np.empty(N, np.int32)
    indeg = np.bincount(dst, minlength=N)
    for c in range(n_cores):
        ids = np.arange(c * NPC, (c + 1) * NPC)
        w, p = _snake_pack(ids, indeg[ids].astype(np.int64), W)
        win_of[ids] = w
        pos_of[ids] = p
        phys[ids] = c * NSLOT + w * P + p

    # per-core edge partitioning and chunking
    # first sizing pass: K = max chunks per window
    edge_core = core_of
    K = 1
    win_edge_counts = np.zeros((n_cores, W), np.int64)
    for c in range(n_cores):
        m = edge_core == c
        wc = np.bincount(win_of[dst[m]], minlength=W)
        win_edge_counts[c] = wc
    K = max(1, int(np.ceil(win_edge_counts.max() / P)))
    NCHUNK = W * K
    ES = NCHUNK * P

    # unique srcs per core (phys rows), 4 ranges
    RT = NSLOT * n_cores // 4  # range size in phys rows
    u_per_core = []
    ncounts = np.zeros((n_cores, 4), np.int64)
    for c in range(n_cores):
        m = edge_core == c
        u = np.unique(phys[src[m]])
        u_per_core.append(u)
        for k in range(4):
            ncounts[c, k] = ((u >= k * RT) & (u < (k + 1) * RT)).sum()
    SAK = int(np.ceil(max(ncounts.max(), 128) / 128)) * 128
    UC = 4 * SAK
    assert UC <= 32760, f"compact table too large: {UC}"

    def wrap16_plan(vals, plan):
        """Layout int16 idx array for dma_gather: per call segment of size L,
        idx j lands at [j%16, col0 + j//16]."""
        v = np.asarray(vals, np.int16)
        segs = []
        n = len(v) // (plan[-1][0] + plan[-1][1])  # number of plan repeats
        per = plan[-1][0] + plan[-1][1]
        for r in range(n):
            for off, L in plan:
                seg = v[r * per + off : r * per + off + L]
                segs.append(seg.reshape(L // 16, 16).T)
        out = np.concatenate(segs, axis=1)
        return np.tile(out, (8, 1))  # replicate across 8 gpsimd cores

    per_core = []
    inv_phys = np.full(NSLOT * n_cores, -1, np.int64)
    for c in range(n_cores):
        ids = np.arange(c * NPC, (c + 1) * NPC)
        inv_phys[phys[ids]] = ids  # phys row -> node id

    for c in range(n_cores):
        m = np.flatnonzero(edge_core == c)
        e_src, e_dst, e_norm = src[m], dst[m], norm_e[m]
        e_win = win_of[e_dst]
        order = np.argsort(e_win, kind="stable")
        e_src, e_dst, e_norm, e_win = (
            e_src[order],
            e_dst[order],
            e_norm[order],
            e_win[order],
        )
        wc = np.bincount(e_win, minlength=W)
        assert wc.max() <= K * P

        u = u_per_core[c]
        # compact ids
        sidx_list = []
        comp_base = {}
        for k in range(4):
            uk = u[(u >= k * RT) & (u < (k + 1) * RT)]
            pad = np.zeros(SAK, np.int64)
            pad[: len(uk)] = uk - k * RT
            sidx_list.append(pad)
            comp_base[k] = (k * SAK, uk)
        sidx_flat = np.concatenate(sidx_list)
        # phys -> compact id lookup (sparse; use searchsorted per range)
        def compact_id(pv):
            k = pv // RT
            out = np.empty(len(pv), np.int64)
            for kk in range(4):
                mk = k == kk
                base, uk = comp_base[kk]
                out[mk] = base + np.searchsorted(uk, pv[mk])
            return out

        # xc table: rows = compact layout
        xc = np.zeros((UC, IN_DIM), np.float32)
        for k in range(4):
            base, uk = comp_base[k]
            xc[base : base + len(uk)] = np.asarray(x)[inv_phys[uk]]

        # per-window edge slots
        gidx_slots = np.zeros(ES, np.int64)  # compact id per slot
        dstloc = np.zeros(ES, np.float32)
        nrm = np.zeros(ES, np.float32)
        off = np.concatenate([[0], np.cumsum(wc)])
        cids = compact_id(phys[e_src])
        for w in range(W):
            a, b = off[w], off[w + 1]
            sl = w * K * P
            nw = b - a
            gidx_slots[sl : sl + nw] = cids[a:b]
            dstloc[sl : sl + nw] = pos_of[e_dst[a:b]]
            nrm[sl : sl + nw] = e_norm[a:b]

        # dn array [2, NCHUNK, P] -> [2, ES] with (c p) order
        dn = np.stack(
            [dstloc.reshape(NCHUNK, P), nrm.reshape(NCHUNK, P)]
        ).reshape(2, ES)

        # per-slot node metadata (window-permuted own nodes)
        ids = np.arange(c * NPC, (c + 1) * NPC)
        slot_node = np.full(NSLOT, -1, np.int64)
        slot_node[win_of[ids] * P + pos_of[ids]] = ids
        valid = slot_node >= 0
        dsq = np.zeros(NSLOT, np.float32)
        dsq[valid] = selfw[slot_node[valid]]
        xp = np.zeros((NSLOT, IN_DIM), np.float32)
        xp[valid] = np.asarray(x)[slot_node[valid]]
        bvals = np.full(NSLOT, -1.0, np.float32)
        icn = np.zeros(NSLOT, np.float32)
        gb_c = int(batch[c * NPC])
        bvals[valid] = (batch[slot_node[valid]] - gb_c).astype(np.float32)
        icn[valid] = icnt_g[batch[slot_node[valid]]]

        per_core.append(
            dict(
                xc=xc,
                xp=xp,
                gidx=None,  # filled after EB known
                _gidx_slots=gidx_slots,
                dn=dn,
                dsq=dsq,
                bloc=bvals,
                icnt=icn,
                sidx=wrap16_plan(sidx_flat, _sa_calls(SAK)),
                gb=gb_c,
            )
        )

    gbs = [pc["gb"] for pc in per_core]
    gspan = max(
        int(batch[(c + 1) * NPC - 1]) - gbs[c] + 1 for c in range(n_cores)
    )
    GW = int(np.ceil(gspan / P)) * P
    assert GW <= 512, GW

    # gather batches are capped at 1024 idxs (64 descriptors/engine packet)
    NBW = 1
    for cand in (4, 2):
        if W % cand == 0 and cand * K * 128 <= 1024:
            NBW = cand
            break
    if NBW == 1 and K * 128 > 1024:
        raise AssertionError(f"K={K} too large for single gather batch")
    meta = dict(
        N=N, E=E, NPC=NPC, W=W, NSLOT=NSLOT, K=K, NCHUNK=NCHUNK, ES=ES,
        RT=RT, SAK=SAK, UC=UC, GW=GW, gbs=gbs, n_graphs=n_graphs,
        n_cores=n_cores, NBW=NBW,
    )
    return per_core, meta


# ------------------------------------------------------------- device kernel


def _build_program(meta):
    import concourse.bacc as bacc
    import concourse.bass as bass
    import concourse.mybir as mybir
    import concourse.tile as tile

    f32 = mybir.dt.float32
    i16 = mybir.dt.int16
    i32 = mybir.dt.int32
    Alu = mybir.AluOpType
    Act = mybir.ActivationFunctionType

    W, K, ES, NSLOT = meta["W"], meta["K"], meta["ES"], meta["NSLOT"]
    NCHUNK, SAK, UC, GW = meta["NCHUNK"], meta["SAK"], meta["UC"], meta["GW"]
    RT, gbs = meta["RT"], meta["gbs"]
    G = meta["n_graphs"]
    n_cores = meta["n_cores"]
    NBLK = math.ceil(G / P)
    GLOBW = max(NBLK * P, max(gbs) + GW)
    NBW = meta["NBW"]  # windows per gather batch
    EB = NBW * K * P  # edge slots per gather batch

    nc = bacc.Bacc(None, target_bir_lowering=False)

    ext_in = {}
    for name, shape, dt in [
        ("xc", [UC, IN_DIM], f32),
        ("xp", [NSLOT, IN_DIM], f32),
        ("gidx", [P, ES // 16], i16),
        ("dn", [2, ES], f32),
        ("dsq", [NSLOT], f32),
        ("bloc", [NSLOT], f32),
        ("icnt", [NSLOT], f32),
        ("sidx", [P, 4 * SAK // 16], i16),
        ("w1", [IN_DIM, HID], f32),
        ("w2", [HID, HID], f32),
        ("wf", [HID, 1], f32),
        ("b1", [1, HID], f32),
        ("b2", [1, HID], f32),
        ("bf", [1, 1], f32),
    ]:
        ext_in[name] = nc.dram_tensor(name, shape, dt, kind="ExternalInput")
    out_ext = nc.dram_tensor("out", [G, 1], f32, kind="ExternalOutput")

    h1_local = nc.dram_tensor("h1_local", [NSLOT, HID], f32)
    h_full = nc.dram_tensor("h_full", [NSLOT * n_cores, HID], f32,
                            addr_space="Shared")
    h_compact = nc.dram_tensor("h_compact", [UC, HID], f32)
    pool_part = nc.dram_tensor("pool_part", [HID, GW], f32)
    pool_all = nc.dram_tensor("pool_all", [HID * n_cores, GW], f32,
                              addr_space="Shared")

    core_ids = list(range(n_cores))

    with ExitStack() as ctx:
        tc = ctx.enter_context(tile.TileContext(nc, num_cores=n_cores))
        cst = ctx.enter_context(tc.tile_pool(name="cst", bufs=1))
        sbw = ctx.enter_context(tc.tile_pool(name="sbw", bufs=3))
        xgp = ctx.enter_context(tc.tile_pool(name="xgp", bufs=2))
        hp = ctx.enter_context(tc.tile_pool(name="hp", bufs=3))
        ps_agg = ctx.enter_context(
            tc.tile_pool(name="ps_agg", bufs=3, space="PSUM"))
        ps_tr = ctx.enter_context(
            tc.tile_pool(name="ps_tr", bufs=2, space="PSUM"))
        ps_pool = ctx.enter_context(
            tc.tile_pool(name="ps_pool", bufs=1, space="PSUM"))

        # ---- constants / metadata loads
        gidx_t = cst.tile([P, ES // 16], i16)
        nc.sync.dma_start(out=gidx_t[:], in_=ext_in["gidx"][:, :])
        sidx_t = cst.tile([P, 4 * SAK // 16], i16)
        nc.sync.dma_start(out=sidx_t[:], in_=ext_in["sidx"][:, :])
        dn_t = cst.tile([P, 2 * NCHUNK], f32)
        nc.sync.dma_start(
            out=dn_t[:], in_=ext_in["dn"].rearrange("t (c p) -> p (t c)", p=P))
        dsq_t = cst.tile([P, W], f32)
        nc.sync.dma_start(
            out=dsq_t[:], in_=ext_in["dsq"].rearrange("(w p) -> p w", p=P))
        bloc_t = cst.tile([P, W], f32)
        nc.sync.dma_start(
            out=bloc_t[:], in_=ext_in["bloc"].rearrange("(w p) -> p w", p=P))
        icnt_t = cst.tile([P, W], f32)
        nc.sync.dma_start(
            out=icnt_t[:], in_=ext_in["icnt"].rearrange("(w p) -> p w", p=P))
        w1_t = cst.tile([IN_DIM, HID], f32)
        nc.sync.dma_start(out=w1_t[:], in_=ext_in["w1"][:, :])
        w2_t = cst.tile([P, 2 * HID], f32)  # W2 K-halves side by side
        nc.sync.dma_start(
            out=w2_t[:].rearrange("k (s h) -> k s h", s=2),
            in_=ext_in["w2"].rearrange("(s k) h -> k s h", k=P))
        wf_t = cst.tile([P, 2], f32)  # wf halves: [:, 0], [:, 1]
        nc.sync.dma_start(
            out=wf_t[:].rearrange("k (s o) -> k s o", s=2),
            in_=ext_in["wf"].rearrange("(s k) o -> k s o", k=P))
        b1_t = cst.tile([1, HID], f32)
        nc.sync.dma_start(out=b1_t[:], in_=ext_in["b1"][:, :])
        b2_t = cst.tile([1, HID], f32)
        nc.sync.dma_start(out=b2_t[:], in_=ext_in["b2"][:, :])
        bf_t = cst.tile([1, 1], f32)
        nc.sync.dma_start(out=bf_t[:], in_=ext_in["bf"][:, :])

        ones_t = cst.tile([1, P], f32)
        nc.vector.memset(ones_t[:], 1.0)

        iota_i = cst.tile([P, K * P], i32)
        nc.gpsimd.iota(iota_i[:], pattern=[[0, K], [1, P]], base=0,
                       channel_multiplier=0)
        iota_f = cst.tile([P, K * P], f32)
        nc.vector.tensor_copy(out=iota_f[:], in_=iota_i[:])
        iotag_i = cst.tile([P, GW], i32)
        nc.gpsimd.iota(iotag_i[:], pattern=[[1, GW]], base=0,
                       channel_multiplier=0)
        iotag_f = cst.tile([P, GW], f32)
        nc.vector.tensor_copy(out=iotag_f[:], in_=iotag_i[:])
        pcol_i = cst.tile([P, 1], i32)
        nc.gpsimd.iota(pcol_i[:], pattern=[[0, 1]], base=0,
                       channel_multiplier=1)
        pcol_f = cst.tile([P, 1], f32)
        nc.vector.tensor_copy(out=pcol_f[:], in_=pcol_i[:])

        # ---------------- layer 1
        for b in range(W // NBW):
            xg = xgp.tile([P, NBW * K, IN_DIM], f32, tag="xg")
            nc.gpsimd.dma_gather(
                out_ap=xg[:],
                in_ap=ext_in["xc"][:, :],
                idxs_ap=gidx_t[:, b * (EB // 16) : (b + 1) * (EB // 16)],
                num_idxs=EB,
                num_idxs_reg=EB,
                elem_size=IN_DIM,
            )
            for wl in range(NBW):
                w = b * NBW + wl
                aggp = ps_agg.tile([P, P], f32, space="PSUM", tag="aggp")
                for ck in range(K):
                    s_t = sbw.tile([P, P], f32, tag="s_t")
                    cg = w * K + ck
                    nc.vector.tensor_scalar(
                        out=s_t[:],
                        in0=iota_f[:, ck * P : (ck + 1) * P],
                        scalar1=dn_t[:, cg : cg + 1],
                        scalar2=dn_t[:, NCHUNK + cg : NCHUNK + cg + 1],
                        op0=Alu.is_equal,
                        op1=Alu.mult,
                    )
                    nc.tensor.matmul(
                        out=aggp[:],
                        lhsT=xg[:, wl * K + ck, :],
                        rhs=s_t[:],
                        start=(ck == 0),
                        stop=False,
                    )
                # self loops: aggT += xp_w^T @ diag(dsq_w)
                xpw = hp.tile([P, IN_DIM], f32, tag="xpw")
                nc.sync.dma_start(
                    out=xpw[:], in_=ext_in["xp"][w * P : (w + 1) * P, :])
                diag = sbw.tile([P, P], f32, tag="diag")
                nc.vector.tensor_scalar(
                    out=diag[:],
                    in0=iota_f[:, 0:P],
                    scalar1=pcol_f[:, 0:1],
                    scalar2=dsq_t[:, w : w + 1],
                    op0=Alu.is_equal,
                    op1=Alu.mult,
                )
                nc.tensor.matmul(
                    out=aggp[:], lhsT=xpw[:], rhs=diag[:],
                    start=False, stop=True)
                agg_sb = sbw.tile([P, P], f32, tag="agg_sb")
                nc.scalar.copy(out=agg_sb[:], in_=aggp[:])
                # transform: h1 = relu(aggT.T @ W1 + b1)
                hpsum = ps_tr.tile([P, HID], f32, space="PSUM", tag="hpsum")
                nc.tensor.matmul(
                    out=hpsum[:], lhsT=agg_sb[:], rhs=w1_t[:],
                    start=True, stop=False)
                nc.tensor.matmul(
                    out=hpsum[:], lhsT=ones_t[:], rhs=b1_t[:],
                    start=False, stop=True)
                h1w = hp.tile([P, HID], f32, tag="h1w")
                nc.vector.tensor_scalar(
                    out=h1w[:], in0=hpsum[:], scalar1=0.0, scalar2=None,
                    op0=Alu.max)
                nc.sync.dma_start(
                    out=h1_local[w * P : (w + 1) * P, :], in_=h1w[:])

        # ---------------- AllGather h1
        nc.gpsimd.collective_compute(
            "AllGather", Alu.bypass, replica_groups=[core_ids],
            ins=[h1_local[:, :]], outs=[h_full[:, :]])

        # ---------------- stage A: compact h rows
        sa_plan = _sa_calls(SAK)
        for k in range(4):
            for off, L in sa_plan:
                hc = xgp.tile([P, SACALL // P, HID], f32, tag="hc")
                cbase = k * SAK + off
                icol = cbase // 16
                nc.gpsimd.dma_gather(
                    out_ap=hc[:, : L // P, :],
                    in_ap=h_full[k * RT : (k + 1) * RT, :],
                    idxs_ap=sidx_t[:, icol : icol + L // 16],
                    num_idxs=L,
                    num_idxs_reg=L,
                    elem_size=HID,
                )
                nc.sync.dma_start(
                    out=h_compact[cbase : cbase + L, :].rearrange(
                        "(c p) d -> p c d", p=P),
                    in_=hc[:, : L // P, :])

        # ---------------- layer 2 + pooling
        poolp = []
        for h in range(2):
            pt = ps_pool.tile([P, GW], f32, space="PSUM", tag=f"poolp{h}")
            poolp.append(pt)
        for b in range(W // NBW):
            hg = xgp.tile([P, NBW * K, HID], f32, tag="hg")
            nc.gpsimd.dma_gather(
                out_ap=hg[:],
                in_ap=h_compact[:, :],
                idxs_ap=gidx_t[:, b * (EB // 16) : (b + 1) * (EB // 16)],
                num_idxs=EB,
                num_idxs_reg=EB,
                elem_size=HID,
            )
            for wl in range(NBW):
                w = b * NBW + wl
                aggp2 = []
                for h in range(2):
                    a2t = ps_agg.tile([P, P], f32, space="PSUM", tag="aggp")
                    aggp2.append(a2t)
                for ck in range(K):
                    s_t = sbw.tile([P, P], f32, tag="s_t")
                    cg = w * K + ck
                    nc.vector.tensor_scalar(
                        out=s_t[:],
                        in0=iota_f[:, ck * P : (ck + 1) * P],
                        scalar1=dn_t[:, cg : cg + 1],
                        scalar2=dn_t[:, NCHUNK + cg : NCHUNK + cg + 1],
                        op0=Alu.is_equal,
                        op1=Alu.mult,
                    )
                    for h in range(2):
                        nc.tensor.matmul(
                            out=aggp2[h][:],
                            lhsT=hg[:, wl * K + ck, h * P : (h + 1) * P],
                            rhs=s_t[:],
                            start=(ck == 0),
                            stop=False,
                        )
                how = hp.tile([P, HID], f32, tag="how")
                nc.sync.dma_start(
                    out=how[:], in_=h1_local[w * P : (w + 1) * P, :])
                diag = sbw.tile([P, P], f32, tag="diag")
                nc.vector.tensor_scalar(
                    out=diag[:],
                    in0=iota_f[:, 0:P],
                    scalar1=pcol_f[:, 0:1],
                    scalar2=dsq_t[:, w : w + 1],
                    op0=Alu.is_equal,
                    op1=Alu.mult,
                )
                for h in range(2):
                    nc.tensor.matmul(
                        out=aggp2[h][:],
                        lhsT=how[:, h * P : (h + 1) * P],
                        rhs=diag[:],
                        start=False,
                        stop=True,
                    )
                hpsum = ps_tr.tile([P, HID], f32, space="PSUM", tag="hpsum")
                for h in range(2):
                    agg_sb = sbw.tile([P, P], f32, tag=f"agg2sb{h}")
                    nc.scalar.copy(out=agg_sb[:], in_=aggp2[h][:])
                    nc.tensor.matmul(
                        out=hpsum[:],
                        lhsT=agg_sb[:],
                        rhs=w2_t[:, h * HID : (h + 1) * HID],
                        start=(h == 0),
                        stop=False,
                    )
                nc.tensor.matmul(
                    out=hpsum[:], lhsT=ones_t[:], rhs=b2_t[:],
                    start=False, stop=True)
                # h2s = relu(h2) * icnt
                h2s = hp.tile([P, HID], f32, tag="h2s")
                nc.vector.tensor_scalar(
                    out=h2s[:], in0=hpsum[:], scalar1=0.0,
                    scalar2=icnt_t[:, w : w + 1], op0=Alu.max, op1=Alu.mult)
                # pooling mask + matmuls
                mask = sbw.tile([P, GW], f32, tag="mask")
                nc.vector.tensor_scalar(
                    out=mask[:], in0=iotag_f[:], scalar1=bloc_t[:, w : w + 1],
                    scalar2=None, op0=Alu.is_equal)
                for h in range(2):
                    nc.tensor.matmul(
                        out=poolp[h][:],
                        lhsT=h2s[:, h * P : (h + 1) * P],
                        rhs=mask[:],
                        start=(w == 0),
                        stop=(w == W - 1),
                    )

        # ---------------- pooled shards out + AllGather
        for h in range(2):
            pp = hp.tile([P, GW], f32, tag="ppsb")
            nc.scalar.copy(out=pp[:], in_=poolp[h][:])
            nc.sync.dma_start(
                out=pool_part[h * P : (h + 1) * P, :], in_=pp[:])
        nc.gpsimd.collective_compute(
            "AllGather", Alu.bypass, replica_groups=[core_ids],
            ins=[pool_part[:, :]], outs=[pool_all[:, :]])

        # ---------------- combine shards into global pooledT
        glob = []
        for h in range(2):
            gt = cst.tile([P, GLOBW], f32, tag=f"glob{h}")
            glob.append(gt)
        for h in range(2):
            nc.vector.memset(glob[h][:], 0.0)
        for r in range(n_cores):
            for h in range(2):
                sh = hp.tile([P, GW], f32, tag="sh")
                nc.sync.dma_start(
                    out=sh[:],
                    in_=pool_all[r * HID + h * P : r * HID + (h + 1) * P, :])
                nc.vector.tensor_tensor(
                    out=glob[h][:, gbs[r] : gbs[r] + GW],
                    in0=glob[h][:, gbs[r] : gbs[r] + GW],
                    in1=sh[:],
                    op=Alu.add,
                )

        # ---------------- final linear + sigmoid
        out_sb = cst.tile([P, NBLK], f32, tag="out_sb")
        for bk in range(NBLK):
            lp = ps_tr.tile([P, 1], f32, space="PSUM", tag="hpsum")
            for h in range(2):
                nc.tensor.matmul(
                    out=lp[:],
                    lhsT=glob[h][:, bk * P : (bk + 1) * P],
                    rhs=wf_t[:, h : h + 1],
                    start=(h == 0),
                    stop=False,
                )
            nc.tensor.matmul(
                out=lp[:], lhsT=ones_t[:], rhs=bf_t[:],
                start=False, stop=True)
            nc.scalar.activation(
                out=out_sb[:, bk : bk + 1], in_=lp[:], func=Act.Sigmoid)
        for bk in range(NBLK):
            cnt = min(P, G - bk * P)
            nc.sync.dma_start(
                out=out_ext[bk * P : bk * P + cnt, :],
                in_=out_sb[:cnt, bk : bk + 1])

    nc.compile()
    return nc, meta


def _layout_gidx(gidx_slots, meta):
    """edge-slot compact ids -> int16 wrapped layout for batched dma_gather."""
    K = meta["K"]
    EB = meta["NBW"] * K * P
    v = np.asarray(gidx_slots, np.int16).reshape(-1, EB)
    cw = EB // 16
    out = np.empty((16, v.size // 16), np.int16)
    for b in range(v.shape[0]):
        out[:, b * cw : (b + 1) * cw] = v[b].reshape(cw, 16).T
    return np.tile(out, (8, 1))


def kernel(x, edge_index, batch, W1, b1, W2, b2, Wf, bf,
           n_graphs=2048, n_cores=8):
    from concourse.bass_utils import run_bass_kernel_spmd

    x = np.asarray(x, np.float32)
    per_core, meta = _prep(x, edge_index, batch, n_graphs, n_cores)
    nc, meta = _build_program(meta)

    w_comm = dict(
        w1=np.asarray(W1, np.float32),
        w2=np.asarray(W2, np.float32),
        wf=np.asarray(Wf, np.float32).reshape(HID, 1),
        b1=np.asarray(b1, np.float32).reshape(1, HID),
        b2=np.asarray(b2, np.float32).reshape(1, HID),
        bf=np.asarray(bf, np.float32).reshape(1, 1),
    )
    in_maps = []
    for pc in per_core:
        m = dict(w_comm)
        m["xc"] = pc["xc"]
        m["xp"] = pc["xp"]
        m["gidx"] = _layout_gidx(pc["_gidx_slots"], meta)
        m["dn"] = pc["dn"]
        m["dsq"] = pc["dsq"]
        m["bloc"] = pc["bloc"]
        m["icnt"] = pc["icnt"]
        m["sidx"] = pc["sidx"]
        in_maps.append(m)

    res = run_bass_kernel_spmd(nc, in_maps, list(range(n_cores)))
    return np.asarray(res.results[0]["out"], np.float32)


# revision 10
# speedup vs baseline: 1.5249x; 1.5249x over previous
"""Distributed 2-layer GCN + mean-pool + linear + sigmoid on 8 TRN2 NeuronCores.

Sharding: nodes (and their in-edges) are sharded across 8 cores by contiguous
dst ranges. Weights replicated. h1 is AllGathered between layers; per-graph
pooled sums are AllGathered and combined on every core.

Aggregation strategy (aggregate-then-transform, GCN is linear so this is
exact): for each 128-node window, gather the x/h rows of in-edge sources
(fast int16 dma_gather from host-compacted tables), build a sparse scatter
matrix S[e, n] = norm_e * onehot(dstloc_e) on DVE (iota + fused is_equal/mult),
and accumulate aggT[d, n] += Xg^T @ S on the TensorEngine.  Self-loops are a
per-window diagonal matmul.  Bias is a K=1 ones-row matmul into the same PSUM
accumulation; relu (and the 1/cnt pooling scale in layer 2) is one fused DVE
tensor_scalar op.
"""

import math
from contextlib import ExitStack

import numpy as np

P = 128
IN_DIM = 128
HID = 256
BF16 = True  # bf16 matmul operands / gather tables (PSUM accum stays fp32)
SACALL = 1024  # max gather call size (64 desc/engine packet limit)


def _sa_calls(SAK):
    """Stage-A call plan: sizes (each %128==0, <=1024) with offsets."""
    plan, off = [], 0
    while off < SAK:
        L = min(SACALL, SAK - off)
        plan.append((off, L))
        off += L
    return plan


# ----------------------------------------------------------------- host prep


def _snake_pack(node_ids, degs, n_bins):
    """Assign nodes to n_bins bins of <=128 nodes, balancing sum of degs.
    Returns window id and position-in-window per node (dense arrays)."""
    order = np.argsort(-degs, kind="stable")
    nb = len(node_ids)
    win = np.empty(nb, np.int32)
    pos = np.empty(nb, np.int32)
    counts = np.zeros(n_bins, np.int32)
    # snake order: 0..n_bins-1, n_bins-1..0, ...
    fwd = np.arange(n_bins)
    snake = np.concatenate([fwd, fwd[::-1]])
    for i, oi in enumerate(order):
        b = snake[i % (2 * n_bins)]
        # bins fill evenly since nb <= 128*n_bins and snake is balanced
        while counts[b] >= P:
            b = (b + 1) % n_bins
        win[oi] = b
        pos[oi] = counts[b]
        counts[b] += 1
    return win, pos


def _prep(x, edge_index, batch, n_graphs, n_cores):
    N = x.shape[0]
    E = edge_index.shape[1]
    NPC = N // n_cores
    W = math.ceil(NPC / P)
    NSLOT = W * P

    src = np.asarray(edge_index[0], np.int64)
    dst = np.asarray(edge_index[1], np.int64)
    batch = np.asarray(batch, np.int64)

    deg = (np.bincount(dst, minlength=N) + 1).astype(np.float32)
    dinv = (1.0 / np.sqrt(deg)).astype(np.float32)
    norm_e = (dinv[src] * dinv[dst]).astype(np.float32)
    selfw = (dinv * dinv).astype(np.float32)
    cnt = np.bincount(batch, minlength=n_graphs).astype(np.float32)
    icnt_g = (1.0 / np.maximum(cnt, 1.0)).astype(np.float32)

    core_of = dst // NPC
    core_of_node = np.arange(N) // NPC

    # pass 1: per-core window packing of own nodes -> global slot map
    phys = np.empty(N, np.int64)  # node -> row in h_full
    win_of = np.empty(N, np.int32)
    pos_of = np.empty(N, np.int32)
    indeg = np.bincount(dst, minlength=N)
    for c in range(n_cores):
        ids = np.arange(c * NPC, (c + 1) * NPC)
        w, p = _snake_pack(ids, indeg[ids].astype(np.int64), W)
        win_of[ids] = w
        pos_of[ids] = p
        phys[ids] = c * NSLOT + w * P + p

    # per-core edge partitioning and chunking
    # first sizing pass: K = max chunks per window
    edge_core = core_of
    K = 1
    win_edge_counts = np.zeros((n_cores, W), np.int64)
    for c in range(n_cores):
        m = edge_core == c
        wc = np.bincount(win_of[dst[m]], minlength=W)
        win_edge_counts[c] = wc
    K = max(1, int(np.ceil(win_edge_counts.max() / P)))
    NCHUNK = W * K
    ES = NCHUNK * P

    # unique srcs per core (phys rows), 4 ranges
    RT = NSLOT * n_cores // 4  # range size in phys rows
    u_per_core = []
    ncounts = np.zeros((n_cores, 4), np.int64)
    for c in range(n_cores):
        m = edge_core == c
        u = np.unique(phys[src[m]])
        u_per_core.append(u)
        for k in range(4):
            ncounts[c, k] = ((u >= k * RT) & (u < (k + 1) * RT)).sum()
    SAK = int(np.ceil(max(ncounts.max(), 128) / 128)) * 128
    UC = 4 * SAK
    assert UC <= 32760, f"compact table too large: {UC}"

    def wrap16_plan(vals, plan):
        """Layout int16 idx array for dma_gather: per call segment of size L,
        idx j lands at [j%16, col0 + j//16]."""
        v = np.asarray(vals, np.int16)
        segs = []
        n = len(v) // (plan[-1][0] + plan[-1][1])  # number of plan repeats
        per = plan[-1][0] + plan[-1][1]
        for r in range(n):
            for off, L in plan:
                seg = v[r * per + off : r * per + off + L]
                segs.append(seg.reshape(L // 16, 16).T)
        out = np.concatenate(segs, axis=1)
        return np.tile(out, (8, 1))  # replicate across 8 gpsimd cores

    per_core = []
    inv_phys = np.full(NSLOT * n_cores, -1, np.int64)
    for c in range(n_cores):
        ids = np.arange(c * NPC, (c + 1) * NPC)
        inv_phys[phys[ids]] = ids  # phys row -> node id

    for c in range(n_cores):
        m = np.flatnonzero(edge_core == c)
        e_src, e_dst, e_norm = src[m], dst[m], norm_e[m]
        e_win = win_of[e_dst]
        order = np.argsort(e_win, kind="stable")
        e_src, e_dst, e_norm, e_win = (
            e_src[order],
            e_dst[order],
            e_norm[order],
            e_win[order],
        )
        wc = np.bincount(e_win, minlength=W)
        assert wc.max() <= K * P

        u = u_per_core[c]
        # compact ids
        sidx_list = []
        comp_base = {}
        for k in range(4):
            uk = u[(u >= k * RT) & (u < (k + 1) * RT)]
            pad = np.zeros(SAK, np.int64)
            pad[: len(uk)] = uk - k * RT
            sidx_list.append(pad)
            comp_base[k] = (k * SAK, uk)
        sidx_flat = np.concatenate(sidx_list)
        # phys -> compact id lookup (sparse; use searchsorted per range)
        def compact_id(pv):
            k = pv // RT
            out = np.empty(len(pv), np.int64)
            for kk in range(4):
                mk = k == kk
                base, uk = comp_base[kk]
                out[mk] = base + np.searchsorted(uk, pv[mk])
            return out

        # xc table: rows = compact layout
        import ml_dtypes
        cdt = ml_dtypes.bfloat16 if BF16 else np.float32
        xc = np.zeros((UC, IN_DIM), cdt)
        for k in range(4):
            base, uk = comp_base[k]
            xc[base : base + len(uk)] = np.asarray(x)[inv_phys[uk]]

        # per-window edge slots
        gidx_slots = np.zeros(ES, np.int64)  # compact id per slot
        dstloc = np.zeros(ES, np.float32)
        nrm = np.zeros(ES, np.float32)
        off = np.concatenate([[0], np.cumsum(wc)])
        cids = compact_id(phys[e_src])
        for w in range(W):
            a, b = off[w], off[w + 1]
            sl = w * K * P
            nw = b - a
            gidx_slots[sl : sl + nw] = cids[a:b]
            dstloc[sl : sl + nw] = pos_of[e_dst[a:b]]
            nrm[sl : sl + nw] = e_norm[a:b]

        # dn array [2, NCHUNK, P] -> [2, ES] with (c p) order
        dn = np.stack(
            [dstloc.reshape(NCHUNK, P), nrm.reshape(NCHUNK, P)]
        ).reshape(2, ES)

        # per-slot node metadata (window-permuted own nodes)
        ids = np.arange(c * NPC, (c + 1) * NPC)
        slot_node = np.full(NSLOT, -1, np.int64)
        slot_node[win_of[ids] * P + pos_of[ids]] = ids
        valid = slot_node >= 0
        dsq = np.zeros(NSLOT, np.float32)
        dsq[valid] = selfw[slot_node[valid]]
        xp = np.zeros((NSLOT, IN_DIM), cdt)
        xp[valid] = np.asarray(x)[slot_node[valid]]
        bvals = np.full(NSLOT, -1.0, np.float32)
        icn = np.zeros(NSLOT, np.float32)
        gb_c = int(batch[c * NPC])
        bvals[valid] = (batch[slot_node[valid]] - gb_c).astype(np.float32)
        icn[valid] = icnt_g[batch[slot_node[valid]]]

        per_core.append(
            dict(
                xc=xc,
                xp=xp,
                gidx=None,  # filled after EB known
                _gidx_slots=gidx_slots,
                dn=dn,
                dsq=dsq,
                bloc=bvals,
                icnt=icn,
                sidx=wrap16_plan(sidx_flat, _sa_calls(SAK)),
                gb=gb_c,
            )
        )

    gbs = [pc["gb"] for pc in per_core]
    gspan = max(
        int(batch[(c + 1) * NPC - 1]) - gbs[c] + 1 for c in range(n_cores)
    )
    GW = int(np.ceil(gspan / P)) * P
    assert GW <= 512, GW

    # gather batches are capped at 1024 idxs (64 descriptors/engine packet)
    NBW = 1
    for cand in (4, 2):
        if W % cand == 0 and cand * K * 128 <= 1024:
            NBW = cand
            break
    if NBW == 1 and K * 128 > 1024:
        raise AssertionError(f"K={K} too large for single gather batch")
    meta = dict(
        N=N, E=E, NPC=NPC, W=W, NSLOT=NSLOT, K=K, NCHUNK=NCHUNK, ES=ES,
        RT=RT, SAK=SAK, UC=UC, GW=GW, gbs=gbs, n_graphs=n_graphs,
        n_cores=n_cores, NBW=NBW,
    )
    return per_core, meta


# ------------------------------------------------------------- device kernel


def _build_program(meta):
    import concourse.bacc as bacc
    import concourse.bass as bass
    import concourse.mybir as mybir
    import concourse.tile as tile

    f32 = mybir.dt.float32
    cdt = mybir.dt.bfloat16 if BF16 else mybir.dt.float32
    i16 = mybir.dt.int16
    i32 = mybir.dt.int32
    Alu = mybir.AluOpType
    Act = mybir.ActivationFunctionType

    W, K, ES, NSLOT = meta["W"], meta["K"], meta["ES"], meta["NSLOT"]
    NCHUNK, SAK, UC, GW = meta["NCHUNK"], meta["SAK"], meta["UC"], meta["GW"]
    RT, gbs = meta["RT"], meta["gbs"]
    G = meta["n_graphs"]
    n_cores = meta["n_cores"]
    NBLK = math.ceil(G / P)
    GLOBW = max(NBLK * P, max(gbs) + GW)
    NBW = meta["NBW"]  # windows per gather batch
    EB = NBW * K * P  # edge slots per gather batch

    nc = bacc.Bacc(None, target_bir_lowering=False)

    ext_in = {}
    for name, shape, dt in [
        ("xc", [UC, IN_DIM], cdt),
        ("xp", [NSLOT, IN_DIM], cdt),
        ("gidx", [P, ES // 16], i16),
        ("dn", [2, ES], f32),
        ("dsq", [NSLOT], f32),
        ("bloc", [NSLOT], f32),
        ("icnt", [NSLOT], f32),
        ("sidx", [P, 4 * SAK // 16], i16),
        ("w1", [IN_DIM, HID], cdt),
        ("w2", [HID, HID], cdt),
        ("wf", [HID, 1], f32),
        ("b1", [1, HID], cdt),
        ("b2", [1, HID], cdt),
        ("bf", [1, 1], f32),
    ]:
        ext_in[name] = nc.dram_tensor(name, shape, dt, kind="ExternalInput")
    out_ext = nc.dram_tensor("out", [G, 1], f32, kind="ExternalOutput")

    h1_local = nc.dram_tensor("h1_local", [NSLOT, HID], cdt)
    h_full = nc.dram_tensor("h_full", [NSLOT * n_cores, HID], cdt,
                            addr_space="Shared")
    h_compact = nc.dram_tensor("h_compact", [UC, HID], cdt)
    pool_part = nc.dram_tensor("pool_part", [HID, GW], f32)
    pool_all = nc.dram_tensor("pool_all", [HID * n_cores, GW], f32,
                              addr_space="Shared")

    core_ids = list(range(n_cores))

    with ExitStack() as ctx:
        tc = ctx.enter_context(tile.TileContext(nc, num_cores=n_cores))
        cst = ctx.enter_context(tc.tile_pool(name="cst", bufs=1))
        sbw = ctx.enter_context(tc.tile_pool(name="sbw", bufs=3))
        xgp = ctx.enter_context(tc.tile_pool(name="xgp", bufs=2))
        hp = ctx.enter_context(tc.tile_pool(name="hp", bufs=3))
        ps_agg = ctx.enter_context(
            tc.tile_pool(name="ps_agg", bufs=3, space="PSUM"))
        ps_tr = ctx.enter_context(
            tc.tile_pool(name="ps_tr", bufs=2, space="PSUM"))
        ps_pool = ctx.enter_context(
            tc.tile_pool(name="ps_pool", bufs=1, space="PSUM"))

        # ---- constants / metadata loads
        gidx_t = cst.tile([P, ES // 16], i16)
        nc.sync.dma_start(out=gidx_t[:], in_=ext_in["gidx"][:, :])
        sidx_t = cst.tile([P, 4 * SAK // 16], i16)
        nc.sync.dma_start(out=sidx_t[:], in_=ext_in["sidx"][:, :])
        dn_t = cst.tile([P, 2 * NCHUNK], f32)
        nc.sync.dma_start(
            out=dn_t[:], in_=ext_in["dn"].rearrange("t (c p) -> p (t c)", p=P))
        dsq_t = cst.tile([P, W], f32)
        nc.sync.dma_start(
            out=dsq_t[:], in_=ext_in["dsq"].rearrange("(w p) -> p w", p=P))
        bloc_t = cst.tile([P, W], f32)
        nc.sync.dma_start(
            out=bloc_t[:], in_=ext_in["bloc"].rearrange("(w p) -> p w", p=P))
        icnt_t = cst.tile([P, W], f32)
        nc.sync.dma_start(
            out=icnt_t[:], in_=ext_in["icnt"].rearrange("(w p) -> p w", p=P))
        w1_t = cst.tile([IN_DIM, HID], cdt)
        nc.sync.dma_start(out=w1_t[:], in_=ext_in["w1"][:, :])
        w2_t = cst.tile([P, 2 * HID], cdt)  # W2 K-halves side by side
        nc.sync.dma_start(
            out=w2_t[:].rearrange("k (s h) -> k s h", s=2),
            in_=ext_in["w2"].rearrange("(s k) h -> k s h", k=P))
        wf_t = cst.tile([P, 2], f32)  # wf halves: [:, 0], [:, 1]
        nc.sync.dma_start(
            out=wf_t[:].rearrange("k (s o) -> k s o", s=2),
            in_=ext_in["wf"].rearrange("(s k) o -> k s o", k=P))
        b1_t = cst.tile([1, HID], cdt)
        nc.sync.dma_start(out=b1_t[:], in_=ext_in["b1"][:, :])
        b2_t = cst.tile([1, HID], cdt)
        nc.sync.dma_start(out=b2_t[:], in_=ext_in["b2"][:, :])
        bf_t = cst.tile([1, 1], f32)
        nc.sync.dma_start(out=bf_t[:], in_=ext_in["bf"][:, :])

        ones_t = cst.tile([1, P], cdt)
        nc.vector.memset(ones_t[:], 1.0)
        ones_f = cst.tile([1, P], f32)
        nc.vector.memset(ones_f[:], 1.0)

        iota_i = cst.tile([P, K * P], i32)
        nc.gpsimd.iota(iota_i[:], pattern=[[0, K], [1, P]], base=0,
                       channel_multiplier=0)
        iota_f = cst.tile([P, K * P], f32)
        nc.vector.tensor_copy(out=iota_f[:], in_=iota_i[:])
        iotag_i = cst.tile([P, GW], i32)
        nc.gpsimd.iota(iotag_i[:], pattern=[[1, GW]], base=0,
                       channel_multiplier=0)
        iotag_f = cst.tile([P, GW], f32)
        nc.vector.tensor_copy(out=iotag_f[:], in_=iotag_i[:])
        pcol_i = cst.tile([P, 1], i32)
        nc.gpsimd.iota(pcol_i[:], pattern=[[0, 1]], base=0,
                       channel_multiplier=1)
        pcol_f = cst.tile([P, 1], f32)
        nc.vector.tensor_copy(out=pcol_f[:], in_=pcol_i[:])

        # ---------------- layer 1
        for b in range(W // NBW):
            xg = xgp.tile([P, NBW * K, IN_DIM], cdt, tag="xg")
            nc.gpsimd.dma_gather(
                out_ap=xg[:],
                in_ap=ext_in["xc"][:, :],
                idxs_ap=gidx_t[:, b * (EB // 16) : (b + 1) * (EB // 16)],
                num_idxs=EB,
                num_idxs_reg=EB,
                elem_size=IN_DIM,
            )
            for wl in range(NBW):
                w = b * NBW + wl
                aggp = ps_agg.tile([P, P], f32, space="PSUM", tag="aggp")
                for ck in range(K):
                    s_t = sbw.tile([P, P], cdt, tag="s_t")
                    cg = w * K + ck
                    nc.vector.tensor_scalar(
                        out=s_t[:],
                        in0=iota_f[:, ck * P : (ck + 1) * P],
                        scalar1=dn_t[:, cg : cg + 1],
                        scalar2=dn_t[:, NCHUNK + cg : NCHUNK + cg + 1],
                        op0=Alu.is_equal,
                        op1=Alu.mult,
                    )
                    nc.tensor.matmul(
                        out=aggp[:],
                        lhsT=xg[:, wl * K + ck, :],
                        rhs=s_t[:],
                        start=(ck == 0),
                        stop=False,
                    )
                # self loops: aggT += xp_w^T @ diag(dsq_w)
                xpw = hp.tile([P, IN_DIM], cdt, tag="xpw")
                nc.sync.dma_start(
                    out=xpw[:], in_=ext_in["xp"][w * P : (w + 1) * P, :])
                diag = sbw.tile([P, P], cdt, tag="diag")
                nc.vector.tensor_scalar(
                    out=diag[:],
                    in0=iota_f[:, 0:P],
                    scalar1=pcol_f[:, 0:1],
                    scalar2=dsq_t[:, w : w + 1],
                    op0=Alu.is_equal,
                    op1=Alu.mult,
                )
                nc.tensor.matmul(
                    out=aggp[:], lhsT=xpw[:], rhs=diag[:],
                    start=False, stop=True)
                agg_sb = sbw.tile([P, P], cdt, tag="agg_sb")
                nc.scalar.copy(out=agg_sb[:], in_=aggp[:])
                # transform: h1 = relu(aggT.T @ W1 + b1)
                hpsum = ps_tr.tile([P, HID], f32, space="PSUM", tag="hpsum")
                nc.tensor.matmul(
                    out=hpsum[:], lhsT=agg_sb[:], rhs=w1_t[:],
                    start=True, stop=False)
                nc.tensor.matmul(
                    out=hpsum[:], lhsT=ones_t[:], rhs=b1_t[:],
                    start=False, stop=True)
                h1w = hp.tile([P, HID], cdt, tag="h1w")
                nc.vector.tensor_scalar(
                    out=h1w[:], in0=hpsum[:], scalar1=0.0, scalar2=None,
                    op0=Alu.max)
                nc.sync.dma_start(
                    out=h1_local[w * P : (w + 1) * P, :], in_=h1w[:])

        # ---------------- AllGather h1
        nc.gpsimd.collective_compute(
            "AllGather", Alu.bypass, replica_groups=[core_ids],
            ins=[h1_local[:, :]], outs=[h_full[:, :]])

        # ---------------- stage A: compact h rows
        sa_plan = _sa_calls(SAK)
        for k in range(4):
            for off, L in sa_plan:
                hc = xgp.tile([P, SACALL // P, HID], cdt, tag="hc")
                cbase = k * SAK + off
                icol = cbase // 16
                nc.gpsimd.dma_gather(
                    out_ap=hc[:, : L // P, :],
                    in_ap=h_full[k * RT : (k + 1) * RT, :],
                    idxs_ap=sidx_t[:, icol : icol + L // 16],
                    num_idxs=L,
                    num_idxs_reg=L,
                    elem_size=HID,
                )
                nc.sync.dma_start(
                    out=h_compact[cbase : cbase + L, :].rearrange(
                        "(c p) d -> p c d", p=P),
                    in_=hc[:, : L // P, :])

        # ---------------- layer 2 + pooling
        poolp = []
        for h in range(2):
            pt = ps_pool.tile([P, GW], f32, space="PSUM", tag=f"poolp{h}")
            poolp.append(pt)
        for b in range(W // NBW):
            hg = xgp.tile([P, NBW * K, HID], cdt, tag="hg")
            nc.gpsimd.dma_gather(
                out_ap=hg[:],
                in_ap=h_compact[:, :],
                idxs_ap=gidx_t[:, b * (EB // 16) : (b + 1) * (EB // 16)],
                num_idxs=EB,
                num_idxs_reg=EB,
                elem_size=HID,
            )
            for wl in range(NBW):
                w = b * NBW + wl
                aggp2 = []
                for h in range(2):
                    a2t = ps_agg.tile([P, P], f32, space="PSUM", tag="aggp")
                    aggp2.append(a2t)
                for ck in range(K):
                    s_t = sbw.tile([P, P], cdt, tag="s_t")
                    cg = w * K + ck
                    nc.vector.tensor_scalar(
                        out=s_t[:],
                        in0=iota_f[:, ck * P : (ck + 1) * P],
                        scalar1=dn_t[:, cg : cg + 1],
                        scalar2=dn_t[:, NCHUNK + cg : NCHUNK + cg + 1],
                        op0=Alu.is_equal,
                        op1=Alu.mult,
                    )
                    for h in range(2):
                        nc.tensor.matmul(
                            out=aggp2[h][:],
                            lhsT=hg[:, wl * K + ck, h * P : (h + 1) * P],
                            rhs=s_t[:],
                            start=(ck == 0),
                            stop=False,
                        )
                how = hp.tile([P, HID], cdt, tag="how")
                nc.sync.dma_start(
                    out=how[:], in_=h1_local[w * P : (w + 1) * P, :])
                diag = sbw.tile([P, P], cdt, tag="diag")
                nc.vector.tensor_scalar(
                    out=diag[:],
                    in0=iota_f[:, 0:P],
                    scalar1=pcol_f[:, 0:1],
                    scalar2=dsq_t[:, w : w + 1],
                    op0=Alu.is_equal,
                    op1=Alu.mult,
                )
                for h in range(2):
                    nc.tensor.matmul(
                        out=aggp2[h][:],
                        lhsT=how[:, h * P : (h + 1) * P],
                        rhs=diag[:],
                        start=False,
                        stop=True,
                    )
                hpsum = ps_tr.tile([P, HID], f32, space="PSUM", tag="hpsum")
                for h in range(2):
                    agg_sb = sbw.tile([P, P], cdt, tag=f"agg2sb{h}")
                    nc.scalar.copy(out=agg_sb[:], in_=aggp2[h][:])
                    nc.tensor.matmul(
                        out=hpsum[:],
                        lhsT=agg_sb[:],
                        rhs=w2_t[:, h * HID : (h + 1) * HID],
                        start=(h == 0),
                        stop=False,
                    )
                nc.tensor.matmul(
                    out=hpsum[:], lhsT=ones_t[:], rhs=b2_t[:],
                    start=False, stop=True)
                # h2s = relu(h2) * icnt
                h2s = hp.tile([P, HID], cdt, tag="h2s")
                nc.vector.tensor_scalar(
                    out=h2s[:], in0=hpsum[:], scalar1=0.0,
                    scalar2=icnt_t[:, w : w + 1], op0=Alu.max, op1=Alu.mult)
                # pooling mask + matmuls
                mask = sbw.tile([P, GW], cdt, tag="mask")
                nc.vector.tensor_scalar(
                    out=mask[:], in0=iotag_f[:], scalar1=bloc_t[:, w : w + 1],
                    scalar2=None, op0=Alu.is_equal)
                for h in range(2):
                    nc.tensor.matmul(
                        out=poolp[h][:],
                        lhsT=h2s[:, h * P : (h + 1) * P],
                        rhs=mask[:],
                        start=(w == 0),
                        stop=(w == W - 1),
                    )

        # ---------------- pooled shards out + AllGather
        for h in range(2):
            pp = hp.tile([P, GW], f32, tag="ppsb")
            nc.scalar.copy(out=pp[:], in_=poolp[h][:])
            nc.sync.dma_start(
                out=pool_part[h * P : (h + 1) * P, :], in_=pp[:])
        nc.gpsimd.collective_compute(
            "AllGather", Alu.bypass, replica_groups=[core_ids],
            ins=[pool_part[:, :]], outs=[pool_all[:, :]])

        # ---------------- combine shards into global pooledT
        glob = []
        for h in range(2):
            gt = cst.tile([P, GLOBW], f32, tag=f"glob{h}")
            glob.append(gt)
        for h in range(2):
            nc.vector.memset(glob[h][:], 0.0)
        for r in range(n_cores):
            for h in range(2):
                sh = hp.tile([P, GW], f32, tag="sh")
                nc.sync.dma_start(
                    out=sh[:],
                    in_=pool_all[r * HID + h * P : r * HID + (h + 1) * P, :])
                nc.vector.tensor_tensor(
                    out=glob[h][:, gbs[r] : gbs[r] + GW],
                    in0=glob[h][:, gbs[r] : gbs[r] + GW],
                    in1=sh[:],
                    op=Alu.add,
                )

        # ---------------- final linear + sigmoid
        out_sb = cst.tile([P, NBLK], f32, tag="out_sb")
        for bk in range(NBLK):
            lp = ps_tr.tile([P, 1], f32, space="PSUM", tag="hpsum")
            for h in range(2):
                nc.tensor.matmul(
                    out=lp[:],
                    lhsT=glob[h][:, bk * P : (bk + 1) * P],
                    rhs=wf_t[:, h : h + 1],
                    start=(h == 0),
                    stop=False,
                )
            nc.tensor.matmul(
                out=lp[:], lhsT=ones_f[:], rhs=bf_t[:],
                start=False, stop=True)
            nc.scalar.activation(
                out=out_sb[:, bk : bk + 1], in_=lp[:], func=Act.Sigmoid)
        for bk in range(NBLK):
            cnt = min(P, G - bk * P)
            nc.sync.dma_start(
                out=out_ext[bk * P : bk * P + cnt, :],
                in_=out_sb[:cnt, bk : bk + 1])

    nc.compile()
    return nc, meta


def _layout_gidx(gidx_slots, meta):
    """edge-slot compact ids -> int16 wrapped layout for batched dma_gather."""
    K = meta["K"]
    EB = meta["NBW"] * K * P
    v = np.asarray(gidx_slots, np.int16).reshape(-1, EB)
    cw = EB // 16
    out = np.empty((16, v.size // 16), np.int16)
    for b in range(v.shape[0]):
        out[:, b * cw : (b + 1) * cw] = v[b].reshape(cw, 16).T
    return np.tile(out, (8, 1))


def kernel(x, edge_index, batch, W1, b1, W2, b2, Wf, bf,
           n_graphs=2048, n_cores=8):
    from concourse.bass_utils import run_bass_kernel_spmd

    x = np.asarray(x, np.float32)
    per_core, meta = _prep(x, edge_index, batch, n_graphs, n_cores)
    nc, meta = _build_program(meta)

    import ml_dtypes
    cdt = ml_dtypes.bfloat16 if BF16 else np.float32
    w_comm = dict(
        w1=np.asarray(W1, np.float32).astype(cdt),
        w2=np.asarray(W2, np.float32).astype(cdt),
        wf=np.asarray(Wf, np.float32).reshape(HID, 1),
        b1=np.asarray(b1, np.float32).reshape(1, HID).astype(cdt),
        b2=np.asarray(b2, np.float32).reshape(1, HID).astype(cdt),
        bf=np.asarray(bf, np.float32).reshape(1, 1),
    )
    in_maps = []
    for pc in per_core:
        m = dict(w_comm)
        m["xc"] = pc["xc"]
        m["xp"] = pc["xp"]
        m["gidx"] = _layout_gidx(pc["_gidx_slots"], meta)
        m["dn"] = pc["dn"]
        m["dsq"] = pc["dsq"]
        m["bloc"] = pc["bloc"]
        m["icnt"] = pc["icnt"]
        m["sidx"] = pc["sidx"]
        in_maps.append(m)

    res = run_bass_kernel_spmd(nc, in_maps, list(range(n_cores)))
    return np.asarray(res.results[0]["out"], np.float32)
